# revision 1
# baseline (speedup 1.0000x reference)
"""DOFEN forward kernel for 8x Trainium2 NeuronCores (pure batch data-parallel).

Contract: kernel(**inputs) takes the FULL inputs from setup_inputs() and
returns the FULL [4096, 10] float32 output.

Heavy algebraic folding happens on the host (numpy); the device program is
built with Bass/Tile and executed via bass_utils.run_bass_kernel_spmd on
cores 0..7, each processing a 512-sample batch shard.

Key device-side structure (per core, feature-partition layout [feat, batch]):
  P1: O = tanh(0.5*U + 0.5*b) via fp32 PE matmul; group sums s (bf16) and
      centered squares q = sum((O - m~)^2) with m~ from bf16 sums --
      avoids the catastrophic q - s^2/4 cancellation.
  P2: GN1 rstd r1 = 2*rsqrt(q + 4eps) via ACT ln/exp (one table-set switch).
  P3: xhat = d*r1 (normalized, unit-scale -> bf16-safe), conv2 via
      block-diagonal bf16 matmul, relu(+bias); GN2 the same way inline;
      conv3 on normalized input; ew = exp(c3 + b3).
  MLP: ew -> DRAM, per-forest row-gather (dma_gather), fused
      (LN1-centered Ep@W1) matmul, relu, LN2 stats via one-hot accumulating
      matmuls, rstd2 batch chain, scaled fc2 accumulated over forests in
      PSUM, rank-1 corrections, transpose out.
"""

import os
import sys

for _p in ("/opt/trn_rl_repo", "/root/.axon_site/_ro/trn_rl_repo"):
    if os.path.isdir(_p) and _p not in sys.path:
        sys.path.insert(0, _p)

import numpy as np
import ml_dtypes

import concourse.bass as bass
import concourse.bacc as bacc
import concourse.tile as tile
from concourse import mybir
import concourse.bass_utils as bass_utils
from concourse import library_config

# ---- problem shapes (hardcoded per contest contract) ----
B = 4096
NCOL = 100
NCOND = 64
D = 4
TOTAL = 6400           # n_col * n_cond
G = 1600               # n_rodt groups
NEST = 160
F = 100                # forests
H = 128                # hidden
C = 10                 # classes
EPS = 1e-5
NCORES = 8
BC = B // NCORES       # 512 per core
NT = TOTAL // 128      # 50 feature tiles
NPACK = (NT + 3) // 4  # 13 packed stats tiles (last covers 2 src tiles)
GPAD = NPACK * 128     # 1664 padded rodt rows
GF_CALL = int(os.environ.get("KGF", "4"))   # forests per dma_gather call
NCALLS = F // GF_CALL
POOL_CASTS = os.environ.get("KPOOLCAST", "1") == "1"
LN2C = float(np.log(2.0))

f32 = mybir.dt.float32
f32r = mybir.dt.float32r
bf16 = mybir.dt.float16   # 16-bit activations/weights use fp16 (11-bit mantissa)
i16 = mybir.dt.int16
AF = mybir.ActivationFunctionType
OP = mybir.AluOpType

BF = np.float16


def _host_prep(inputs):
    """Fold all parameter algebra on the host; returns dict of device arrays."""
    f64 = np.float64
    x = np.asarray(inputs["x"], np.float32)
    w1 = np.asarray(inputs["w1"], f64)
    b1 = np.asarray(inputs["b1"], f64)
    perm = np.asarray(inputs["perm"], np.int64)
    gn1_w = np.asarray(inputs["gn1_w"], f64)
    gn1_b = np.asarray(inputs["gn1_b"], f64)
    conv2_w = np.asarray(inputs["conv2_w"], f64)
    conv2_b = np.asarray(inputs["conv2_b"], f64)
    gn2_w = np.asarray(inputs["gn2_w"], f64)
    gn2_b = np.asarray(inputs["gn2_b"], f64)
    conv3_w = np.asarray(inputs["conv3_w"], f64)
    conv3_b = np.asarray(inputs["conv3_b"], f64)
    swr = np.asarray(inputs["swr"], np.int64)
    E = np.asarray(inputs["E"], f64)
    ln1_w = np.asarray(inputs["ln1_w"], f64)
    ln1_b = np.asarray(inputs["ln1_b"], f64)
    fc1_w = np.asarray(inputs["fc1_w"], f64)
    fc1_b = np.asarray(inputs["fc1_b"], f64)
    ln2_w = np.asarray(inputs["ln2_w"], f64)
    ln2_b = np.asarray(inputs["ln2_b"], f64)
    fc2_w = np.asarray(inputs["fc2_w"], f64)
    fc2_b = np.asarray(inputs["fc2_b"], f64)

    dev = {}

    # --- O-matmul weights: feature t corresponds to reference feature perm[t]
    p = perm
    c_arr = p // NCOL
    j_arr = p % NCOL
    What = np.zeros((100, TOTAL), np.float32)
    What[j_arr, np.arange(TOTAL)] = w1[j_arr, c_arr]
    dev["what"] = What.astype(BF)
    # folded into the tanh bias: tanh(0.5*U + 0.5*bhat)
    dev["bhat"] = (0.5 * b1[j_arr, c_arr]).reshape(NT, 128).T.astype(np.float32).copy()

    # --- GN1 affine folded into conv2 weights/bias ---
    w2f = conv2_w * gn1_w.reshape(G, D, 1)                 # [G, D, D]
    b2f = conv2_b.reshape(G, D) + np.einsum("gi,gio->go", gn1_b.reshape(G, D), conv2_w)
    bd2 = np.zeros((128, NT * 128), np.float32)
    gi = np.arange(32)
    for t in range(NT):
        g0 = 32 * t
        for i in range(D):
            for o in range(D):
                bd2[4 * gi + i, t * 128 + 4 * gi + o] = w2f[g0 + gi, i, o]
    dev["bd2"] = bd2.astype(BF)

    # --- GN2 + conv3 ---
    w3f = conv3_w[:, :, 0] * gn2_w.reshape(G, D)
    b3f = conv3_b + (gn2_b.reshape(G, D) * conv3_w[:, :, 0]).sum(1)
    bd3 = np.zeros((128, NT * 32), np.float32)
    for t in range(NT):
        g0 = 32 * t
        for i in range(D):
            bd3[4 * gi + i, t * 32 + gi] = w3f[g0 + gi, i]
    dev["bd3"] = bd3.astype(BF)

    # ones-block for group sums: [128, 32]
    onesblk = np.zeros((128, 32), np.float32)
    onesblk[np.arange(128), np.arange(128) // 4] = 1.0
    dev["onesblk"] = onesblk.astype(BF)

    # group-stat broadcast selectors: bcastj[k, j*128+m] = (k == 32*j + m//4)
    bcastj = np.zeros((128, 4 * 128), np.float32)
    for j in range(4):
        m = np.arange(128)
        bcastj[32 * j + m // 4, j * 128 + m] = 1.0
    dev["bcastj"] = bcastj.astype(BF)
    dev["bcastj25"] = (0.25 * bcastj).astype(BF)

    # bias columns
    b2f_col = np.zeros((128, NT), np.float32)
    pp = np.arange(128)
    for t in range(NT):
        b2f_col[pp, t] = b2f[32 * t + pp // 4, pp % 4]
    dev["b2fcol"] = b2f_col
    b3f_pad = np.zeros(GPAD, np.float64)
    b3f_pad[:G] = b3f
    dev["b3fcol"] = (b3f_pad - 4.0).reshape(NPACK, 128).T.astype(np.float32).copy()

    # --- MLP folds ---
    W1p = ln1_w[:, None] * fc1_w                           # [128, 128]
    b1p = fc1_b + ln1_b @ fc1_w
    if np.abs(b1p).max() > 1e-12:
        raise NotImplementedError(
            "fused kernel assumes fc1_b + ln1_b@fc1_w == 0 (true for this problem)"
        )
    Ep = E[swr]                                            # [F, 160, 128]
    muEp = Ep.mean(2)                                      # [F, 160]
    cs1 = W1p.sum(0)                                       # [128]
    Gall = np.einsum("fed,dh->feh", Ep, W1p) - muEp[:, :, None] * cs1[None, None, :]
    g1 = np.ascontiguousarray(Gall[:, :128, :].transpose(1, 0, 2)).reshape(128, F * H)
    g2 = np.ascontiguousarray(Gall[:, 128:, :].transpose(1, 0, 2)).reshape(32, F * H)
    dev["g1"] = g1.astype(BF)
    dev["g2"] = g2.astype(BF)

    W2p = ln2_w[:, None] * fc2_w                           # [128, 10]
    b2p = fc2_b + ln2_b @ fc2_w                            # [10]
    W2pp = np.sqrt(128.0) * W2p
    cs2 = W2pp.sum(0)
    dev["w2pp"] = W2pp.astype(BF)
    dev["negcs2"] = (-cs2).reshape(1, C).astype(BF)
    dev["b2ppx"] = (float(F) * b2p).reshape(1, C).astype(BF)

    # one-hot columns for LN2 stats accumulation (two 50-row halves)
    FH = F // 2
    ohcol = np.zeros((128, F * FH), np.float32)
    for f in range(F):
        ohcol[:, f * FH + (f % FH)] = 1.0
    dev["ohcol"] = ohcol.astype(BF)

    # selector rows for rstd2 broadcast: selq[k, f*128+m] = (k == f%50)
    FH2 = F // 2
    selq = np.zeros((FH2, F * 128), np.float32)
    for f in range(F):
        selq[f % FH2, f * 128:(f + 1) * 128] = 1.0
    dev["selq"] = selq.astype(BF)

    # gather indices, wrapped in 16 partitions per call of GF_CALL forests
    idx_cols = []
    for call in range(NCALLS):
        L = []
        for f in range(call * GF_CALL, (call + 1) * GF_CALL):
            L.extend(swr[f, :128].tolist())
            L.extend(swr[f, 128:160].tolist())
            L.extend([0] * 96)
        L = np.asarray(L, np.int16)
        wrap = L.reshape(-1, 16).T
        idx_cols.append(np.tile(wrap, (8, 1)))
    dev["gidx"] = np.concatenate(idx_cols, axis=1)  # [128, F*16] int16

    return dev, x


def _build_program():
    """Build the per-core Bass program."""
    nc = bacc.Bacc("TRN2", debug=False, num_devices=NCORES)

    def din(name, shape, dt):
        return nc.dram_tensor(name, list(shape), dt, kind="ExternalInput").ap()

    x_d = din("x_shard", [BC, NCOL], f32)
    what_d = din("what", [100, TOTAL], bf16)
    bhat_d = din("bhat", [128, NT], f32)
    bd2_d = din("bd2", [128, NT * 128], bf16)
    bd3_d = din("bd3", [128, NT * 32], bf16)
    onesblk_d = din("onesblk", [128, 32], bf16)
    bcastj_d = din("bcastj", [128, 4 * 128], bf16)
    bcastj25_d = din("bcastj25", [128, 4 * 128], bf16)
    b2fcol_d = din("b2fcol", [128, NT], f32)
    b3fcol_d = din("b3fcol", [128, NPACK], f32)
    g1_d = din("g1", [128, F * H], bf16)
    g2_d = din("g2", [32, F * H], bf16)
    w2pp_d = din("w2pp", [H, C], bf16)
    negcs2_d = din("negcs2", [1, C], bf16)
    b2ppx_d = din("b2ppx", [1, C], bf16)
    selq_d = din("selq", [F // 2, F * 128], bf16)
    ohcol_d = din("ohcol", [128, (F // 2) * F], bf16)
    gidx_d = din("gidx", [128, F * 16], i16)

    y_d = nc.dram_tensor("y_out", [BC, C], f32, kind="ExternalOutput").ap()

    from contextlib import ExitStack
    from concourse.masks import make_identity

    with tile.TileContext(nc) as tc, ExitStack() as ctx:
        persist = ctx.enter_context(tc.tile_pool(name="persist", bufs=1))
        dram_pool = ctx.enter_context(tc.tile_pool(name="drams", bufs=1, space="DRAM"))

        ident = persist.tile([128, 128], f32)
        make_identity(nc, ident[:])
        onesrow = persist.tile([1, BC], bf16)
        nc.vector.memset(onesrow[:], 1.0)
        eps4 = persist.tile([128, 1], f32)
        nc.vector.memset(eps4[:], 4.0 * EPS)
        eps16 = persist.tile([128, 1], f32)
        nc.vector.memset(eps16[:], 16.0 * EPS)
        epsH = persist.tile([128, 1], f32)
        nc.vector.memset(epsH[:], float(H) * EPS)
        ln2b = persist.tile([128, 1], f32)
        nc.vector.memset(ln2b[:], LN2C)

        w2pp = persist.tile([H, C], bf16)
        nc.sync.dma_start(out=w2pp[:], in_=w2pp_d)
        negcs2 = persist.tile([1, C], bf16)
        nc.sync.dma_start(out=negcs2[:], in_=negcs2_d)
        b2ppx = persist.tile([1, C], bf16)
        nc.sync.dma_start(out=b2ppx[:], in_=b2ppx_d)

        ew_dram = dram_pool.tile([GPAD, BC], bf16)

        # ---------- phi2 ----------
        with ExitStack() as phi_ctx:
            pconst = phi_ctx.enter_context(tc.tile_pool(name="pconst", bufs=1))
            pwork = phi_ctx.enter_context(tc.tile_pool(name="pwork", bufs=2))
            ppack = phi_ctx.enter_context(tc.tile_pool(name="ppack", bufs=2))
            pstat = phi_ctx.enter_context(tc.tile_pool(name="pstat", bufs=2))
            pstat1 = phi_ctx.enter_context(tc.tile_pool(name="pstat1", bufs=1))

            onesblk = pconst.tile([128, 32], bf16)
            nc.sync.dma_start(out=onesblk[:], in_=onesblk_d)
            bcastj = pconst.tile([128, 4 * 128], bf16)
            nc.sync.dma_start(out=bcastj[:], in_=bcastj_d)
            bcastj25 = pconst.tile([128, 4 * 128], bf16)
            nc.sync.dma_start(out=bcastj25[:], in_=bcastj25_d)
            b2fcol = pconst.tile([128, NT], f32)
            nc.sync.dma_start(out=b2fcol[:], in_=b2fcol_d)
            b3fcol = pconst.tile([128, NPACK], f32)
            nc.sync.dma_start(out=b3fcol[:], in_=b3fcol_d)
            bhat_sb = pconst.tile([128, NT], f32)
            nc.sync.dma_start(out=bhat_sb[:], in_=bhat_d)

            ew_sb = pconst.tile([128, NPACK, BC], bf16)
            nc.vector.memset(ew_sb[64:128, NPACK - 1, :], 0.0)

            dall = pconst.tile([128, NT, BC], bf16)
            r1_all = pconst.tile([128, NPACK, BC], bf16)

            sAB = phi_ctx.enter_context(ExitStack())
            qpool = sAB.enter_context(tc.tile_pool(name="qpool", bufs=1))
            qall = qpool.tile([128, NPACK, BC], f32)
            nc.vector.memset(qall[64:128, NPACK - 1, :], 1.0)

            # ---- P1 (ACT set: tanh/copy): O, bf16 sums, centered squares
            with ExitStack() as sA:
                pA = sA.enter_context(tc.tile_pool(name="pA", bufs=1))
                what_sb = pA.tile([100, TOTAL], bf16)
                nc.sync.dma_start(out=what_sb[:], in_=what_d)

                x_aug = pA.tile([100, BC], bf16)
                with tc.tile_pool(name="xtp", bufs=2, space="PSUM") as xtpool:
                    for bt in range(BC // 128):
                        x_t = pwork.tile([128, NCOL], f32, tag="xload")
                        nc.sync.dma_start(out=x_t[:], in_=x_d[bt * 128:(bt + 1) * 128, :])
                        x_ps = xtpool.tile([NCOL, 128], f32, tag="xtps")
                        nc.tensor.transpose(out=x_ps[:], in_=x_t[:], identity=ident[:])
                        nc.vector.tensor_copy(
                            out=x_aug[0:NCOL, bt * 128:(bt + 1) * 128], in_=x_ps[:]
                        )

                popool = sA.enter_context(tc.tile_pool(name="popool", bufs=2, space="PSUM"))
                stA = sA.enter_context(tc.tile_pool(name="stA", bufs=2, space="PSUM"))
                mbA = sA.enter_context(tc.tile_pool(name="mbA", bufs=2, space="PSUM"))


                for st in range(NPACK):
                    tiles = list(range(4 * st, min(4 * st + 4, NT)))
                    nrow = 32 * len(tiles)
                    s_ps = stA.tile([128, BC], f32, tag="s1")
                    q_ps = stA.tile([128, BC], f32, tag="q1")
                    o_tiles = []
                    for t in tiles:
                        j = t % 4
                        po = popool.tile([128, BC], f32, tag="po")
                        nc.tensor.matmul(
                            out=po[:], lhsT=what_sb[:, t * 128:(t + 1) * 128],
                            rhs=x_aug[:], start=True, stop=True,
                        )
                        of = ppack.tile([128, BC], bf16, tag=f"of{j}")
                        nc.scalar.activation(
                            out=of[:], in_=po[:], func=AF.Tanh, scale=0.5,
                            bias=bhat_sb[:, t:t + 1],
                        )
                        o_tiles.append((t, of))
                        nc.tensor.matmul(
                            out=s_ps[32 * j:32 * j + 32, :], lhsT=onesblk[:],
                            rhs=of[:], start=True, stop=True,
                            tile_position=(0, 32 * j),
                        )
                    scp = pstat.tile([128, BC], bf16, tag="scp")
                    nc.scalar.activation(out=scp[:nrow], in_=s_ps[:nrow], func=AF.Copy)
                    for (t, of) in o_tiles:
                        j = t % 4
                        mb_ps = mbA.tile([128, BC], f32, tag="mb")
                        nc.tensor.matmul(
                            out=mb_ps[:], lhsT=bcastj25[0:nrow, j * 128:(j + 1) * 128],
                            rhs=scp[0:nrow], start=True, stop=True,
                        )
                        d = dall[:, t, :]
                        nc.vector.tensor_tensor(
                            out=d, in0=of[:], in1=mb_ps[:], op=OP.subtract
                        )
                        dsq = pwork.tile([128, BC], bf16, tag="dsq")
                        nc.gpsimd.tensor_tensor(out=dsq[:], in0=d, in1=d, op=OP.mult)
                        nc.tensor.matmul(
                            out=q_ps[32 * j:32 * j + 32, :], lhsT=onesblk[:],
                            rhs=dsq[:], start=True, stop=True,
                            tile_position=(0, 32 * j),
                        )
                    nc.scalar.activation(
                        out=qall[:nrow, st, :], in_=q_ps[:nrow], func=AF.Copy
                    )

            # hard barrier: keep all tanh (exp-set) ACT ops before any Ln
            tc.strict_bb_all_engine_barrier()

            # ---- P2: GN1 rstd -- one fused Ln, one fused Exp over all packs
            nc.scalar.activation(
                out=qall[:], in_=qall[:], func=AF.Ln, bias=eps16[:],
            )
            nc.scalar.activation(
                out=r1_all[:], in_=qall[:], func=AF.Exp, scale=-0.5, bias=ln2b[:],
            )

            # ---- P3a: xhat, conv2, relu, GN2 sums & centered squares
            r2_all = pconst.tile([128, NPACK, BC], bf16)
            with ExitStack() as sB:
                pC = sB.enter_context(tc.tile_pool(name="pC", bufs=1))
                bd2_sb = pC.tile([128, NT * 128], bf16)
                nc.sync.dma_start(out=bd2_sb[:], in_=bd2_d)
                bd3_sb = pC.tile([128, NT * 32], bf16)
                nc.sync.dma_start(out=bd3_sb[:], in_=bd3_d)

                wpool = sB.enter_context(tc.tile_pool(name="wpool", bufs=4, space="PSUM"))
                stB = sB.enter_context(tc.tile_pool(name="stB", bufs=2, space="PSUM"))

                for st in range(NPACK):
                    tiles = list(range(4 * st, min(4 * st + 4, NT)))
                    nrow = 32 * len(tiles)
                    s_ps = stB.tile([128, BC], f32, tag="s2")
                    q_ps = stB.tile([128, BC], f32, tag="q2")
                    h_tiles = []
                    for t in tiles:
                        j = t % 4
                        rb_ps = wpool.tile([128, BC], f32, tag="w")
                        nc.tensor.matmul(
                            out=rb_ps[:], lhsT=bcastj[0:nrow, j * 128:(j + 1) * 128],
                            rhs=r1_all[0:nrow, st, :], start=True, stop=True,
                        )
                        xh = pwork.tile([128, BC], bf16, tag="xh")
                        nc.vector.tensor_tensor(
                            out=xh[:], in0=dall[:, t, :], in1=rb_ps[:], op=OP.mult
                        )
                        c2_ps = wpool.tile([128, BC], f32, tag="w")
                        nc.tensor.matmul(
                            out=c2_ps[:], lhsT=bd2_sb[:, t * 128:(t + 1) * 128],
                            rhs=xh[:], start=True, stop=True,
                        )
                        hf = ppack.tile([128, BC], bf16, tag=f"hf{j}")
                        nc.scalar.activation(
                            out=hf[:], in_=c2_ps[:], func=AF.Relu,
                            bias=b2fcol[:, t:t + 1],
                        )
                        h_tiles.append((t, hf))
                        nc.tensor.matmul(
                            out=s_ps[32 * j:32 * j + 32, :], lhsT=onesblk[:],
                            rhs=hf[:], start=True, stop=True,
                            tile_position=(0, 32 * j),
                        )
                    scp2 = pstat.tile([128, BC], bf16, tag="scp2")
                    nc.scalar.activation(out=scp2[:nrow], in_=s_ps[:nrow], func=AF.Copy)
                    for (t, hf) in h_tiles:
                        j = t % 4
                        mb_ps = wpool.tile([128, BC], f32, tag="w")
                        nc.tensor.matmul(
                            out=mb_ps[:], lhsT=bcastj25[0:nrow, j * 128:(j + 1) * 128],
                            rhs=scp2[0:nrow], start=True, stop=True,
                        )
                        d2 = dall[:, t, :]
                        nc.vector.tensor_tensor(
                            out=d2, in0=hf[:], in1=mb_ps[:], op=OP.subtract
                        )
                        d2sq = pwork.tile([128, BC], bf16, tag="d2sq")
                        nc.gpsimd.tensor_tensor(out=d2sq[:], in0=d2, in1=d2, op=OP.mult)
                        nc.tensor.matmul(
                            out=q_ps[32 * j:32 * j + 32, :], lhsT=onesblk[:],
                            rhs=d2sq[:], start=True, stop=True,
                            tile_position=(0, 32 * j),
                        )
                    nc.scalar.activation(
                        out=qall[:nrow, st, :], in_=q_ps[:nrow], func=AF.Copy
                    )

                # ---- P3b/P3c: one fused Ln, one fused Exp
                tc.strict_bb_all_engine_barrier()
                nc.scalar.activation(
                    out=qall[:], in_=qall[:], func=AF.Ln, bias=eps4[:],
                )
                nc.scalar.activation(
                    out=r2_all[:], in_=qall[:], func=AF.Exp, scale=-0.5, bias=ln2b[:],
                )

                # ---- P3d: xhat2, conv3, exp
                for st in range(NPACK):
                    tiles = list(range(4 * st, min(4 * st + 4, NT)))
                    nrow = 32 * len(tiles)
                    c3_ps = wpool.tile([128, BC], f32, tag="w")
                    for t in tiles:
                        j = t % 4
                        rb2_ps = wpool.tile([128, BC], f32, tag="w")
                        nc.tensor.matmul(
                            out=rb2_ps[:], lhsT=bcastj[0:nrow, j * 128:(j + 1) * 128],
                            rhs=r2_all[0:nrow, st, :], start=True, stop=True,
                        )
                        xh2 = pwork.tile([128, BC], bf16, tag="xh2")
                        nc.vector.tensor_tensor(
                            out=xh2[:], in0=dall[:, t, :], in1=rb2_ps[:], op=OP.mult
                        )
                        nc.tensor.matmul(
                            out=c3_ps[32 * j:32 * j + 32, :],
                            lhsT=bd3_sb[:, t * 32:(t + 1) * 32],
                            rhs=xh2[:], start=True, stop=True,
                            tile_position=(0, 32 * j),
                        )
                    nc.scalar.activation(
                        out=ew_sb[:nrow, st, :], in_=c3_ps[:nrow], func=AF.Exp,
                        bias=b3fcol[:nrow, st:st + 1],
                    )

            nc.sync.dma_start(
                out=ew_dram[:].rearrange("(c p) w -> p c w", p=128), in_=ew_sb[:]
            )

        # ---------- MLP ----------
        with ExitStack() as mlp_ctx:
            zkeep = mlp_ctx.enter_context(tc.tile_pool(name="zkeep", bufs=1))
            zall = zkeep.tile([128, F, BC], bf16)
            qsb = zkeep.tile([F // 2, 2, BC], bf16)
            bsum = zkeep.tile([1, BC], bf16)

            # ----- pass 1: gather + fused fc1 + relu (stats inline) -----
            m_ctx = mlp_ctx.enter_context(ExitStack())
            if True:
                mconst = m_ctx.enter_context(tc.tile_pool(name="mconst", bufs=1))
                gpool = m_ctx.enter_context(tc.tile_pool(name="gpool", bufs=2))
                mpsum = m_ctx.enter_context(tc.tile_pool(name="mpsum", bufs=3, space="PSUM"))

                g1_sb = mconst.tile([128, F * H], bf16)
                nc.sync.dma_start(out=g1_sb[:], in_=g1_d)
                g2_sb = mconst.tile([32, F * H], bf16)
                nc.sync.dma_start(out=g2_sb[:], in_=g2_d)
                gidx = mconst.tile([128, F * 16], i16)
                nc.sync.dma_start(out=gidx[:], in_=gidx_d)
                FH = F // 2
                ohcol_sb = mconst.tile([128, F * FH], bf16)
                nc.sync.dma_start(out=ohcol_sb[:], in_=ohcol_d)
                stm = m_ctx.enter_context(tc.tile_pool(name="stm", bufs=1, space="PSUM"))
                s2h0 = stm.tile([FH, BC], f32, tag="s2h0", name="s2h0")
                s2h1 = stm.tile([FH, BC], f32, tag="s2h1", name="s2h1")
                q2h0 = stm.tile([FH, BC], f32, tag="q2h0", name="q2h0")
                q2h1 = stm.tile([FH, BC], f32, tag="q2h1", name="q2h1")
                s2h = [s2h0, s2h1]
                q2h = [q2h0, q2h1]

                for call in range(NCALLS):
                    gout = gpool.tile([128, 2 * GF_CALL, BC], bf16, tag="gout")
                    nidx = GF_CALL * 256
                    nc.gpsimd.dma_gather(
                        out_ap=gout[:],
                        in_ap=ew_dram[:],
                        idxs_ap=gidx[:, call * GF_CALL * 16:(call + 1) * GF_CALL * 16],
                        num_idxs=nidx,
                        num_idxs_reg=nidx,
                        elem_size=BC,
                    )
                    for jf in range(GF_CALL):
                        f = call * GF_CALL + jf
                        z_ps = mpsum.tile([H, BC], f32, tag="zps")
                        nc.tensor.matmul(
                            out=z_ps[:], lhsT=g1_sb[:, f * H:(f + 1) * H],
                            rhs=gout[:, 2 * jf, :], start=True, stop=False,
                        )
                        nc.tensor.matmul(
                            out=z_ps[:], lhsT=g2_sb[:, f * H:(f + 1) * H],
                            rhs=gout[0:32, 2 * jf + 1, :], start=False, stop=True,
                        )
                        z_f = zall[:, f, :]
                        if f % 2 == 0:
                            nc.scalar.activation(out=z_f, in_=z_ps[:], func=AF.Relu)
                        else:
                            nc.vector.tensor_scalar_max(out=z_f, in0=z_ps[:], scalar1=0.0)
                        zsq = gpool.tile([H, BC], bf16, tag="zsq")
                        nc.vector.tensor_tensor(out=zsq[:], in0=z_f, in1=z_f, op=OP.mult)
                        half = f // FH
                        nc.tensor.matmul(
                            out=s2h[half][:], lhsT=ohcol_sb[:, f * FH:(f + 1) * FH],
                            rhs=z_f, start=(f % FH == 0), stop=(f % FH == FH - 1),
                        )
                        nc.tensor.matmul(
                            out=q2h[half][:], lhsT=ohcol_sb[:, f * FH:(f + 1) * FH],
                            rhs=zsq[:], start=(f % FH == 0), stop=(f % FH == FH - 1),
                        )

            # ----- interlude: batched rstd2 from in-pass1 stats -----
            with ExitStack() as i_ctx:
                iwork = i_ctx.enter_context(tc.tile_pool(name="iwork", bufs=1))
                ipsum = i_ctx.enter_context(tc.tile_pool(name="ipsum", bufs=1, space="PSUM"))

                gam = iwork.tile([FH, 2, BC], bf16, tag="gam")
                for i in range(2):
                    scp2 = iwork.tile([FH, BC], f32, tag=f"iscp2_{i}", name=f"iscp2_{i}")
                    nc.scalar.activation(out=scp2[:], in_=s2h[i][:], func=AF.Copy)
                    u2 = iwork.tile([FH, BC], f32, tag=f"u2_{i}", name=f"u2_{i}")
                    nc.vector.scalar_tensor_tensor(
                        out=u2[:], in0=scp2[:], scalar=-1.0 / H, in1=scp2[:],
                        op0=OP.mult, op1=OP.mult,
                    )
                    t2 = iwork.tile([FH, BC], f32, tag=f"t2_{i}", name=f"t2_{i}")
                    nc.vector.tensor_tensor(out=t2[:], in0=u2[:], in1=q2h[i][:], op=OP.add)
                    nc.vector.tensor_scalar_max(out=t2[:], in0=t2[:], scalar1=0.0)
                    nc.scalar.activation(out=t2[:], in_=t2[:], func=AF.Ln, bias=epsH[:FH])
                    nc.scalar.activation(out=qsb[:, i, :], in_=t2[:], func=AF.Exp, scale=-0.5)
                    qf32 = iwork.tile([FH, BC], f32, tag=f"qf32_{i}", name=f"qf32_{i}")
                    nc.vector.tensor_copy(out=qf32[:], in_=qsb[:, i, :])
                    nc.vector.scalar_tensor_tensor(
                        out=gam[:, i, :], in0=scp2[:], scalar=1.0 / H, in1=qf32[:],
                        op0=OP.mult, op1=OP.mult,
                    )
                ones50 = iwork.tile([FH, 1], bf16, tag="o50")
                nc.vector.memset(ones50[:], 1.0)
                bs_ps = ipsum.tile([1, BC], f32, tag="bsps")
                nc.tensor.matmul(
                    out=bs_ps[:], lhsT=ones50[:], rhs=gam[:, 0, :], start=True, stop=False
                )
                nc.tensor.matmul(
                    out=bs_ps[:], lhsT=ones50[:], rhs=gam[:, 1, :], start=False, stop=True
                )
                nc.scalar.activation(out=bsum[:], in_=bs_ps[:], func=AF.Copy)

            m_ctx.close()

            # ----- pass 2: scale z, fc2 accumulate, output -----
            with ExitStack() as p2_ctx:
                p2const = p2_ctx.enter_context(tc.tile_pool(name="p2const", bufs=1))
                p2psum = p2_ctx.enter_context(tc.tile_pool(name="p2psum", bufs=2, space="PSUM"))
                ypsum = p2_ctx.enter_context(tc.tile_pool(name="ypsum", bufs=1, space="PSUM"))
                p2work = p2_ctx.enter_context(tc.tile_pool(name="p2work", bufs=3))

                selq_sb = p2const.tile([F // 2, F * 128], bf16)
                nc.sync.dma_start(out=selq_sb[:], in_=selq_d)

                y_ps = ypsum.tile([C, BC], f32)
                for f in range(F):
                    qb_ps = p2psum.tile([128, BC], f32, tag="qbps")
                    nc.tensor.matmul(
                        out=qb_ps[:], lhsT=selq_sb[:, f * 128:(f + 1) * 128],
                        rhs=qsb[:, f // (F // 2), :], start=True, stop=True,
                    )
                    zsc = p2work.tile([128, BC], bf16, tag="zsc")
                    nc.vector.tensor_tensor(
                        out=zsc[:], in0=zall[:, f, :], in1=qb_ps[:], op=OP.mult
                    )
                    nc.tensor.matmul(
                        out=y_ps[:], lhsT=w2pp[:], rhs=zsc[:],
                        start=(f == 0), stop=False,
                    )
                nc.tensor.matmul(
                    out=y_ps[:], lhsT=negcs2[:], rhs=bsum[:], start=False, stop=False
                )
                nc.tensor.matmul(
                    out=y_ps[:], lhsT=b2ppx[:], rhs=onesrow[:], start=False, stop=True
                )
                ysb = p2work.tile([C, BC], f32, tag="ysb")
                nc.scalar.activation(out=ysb[:], in_=y_ps[:], func=AF.Copy, scale=1.0 / F)
                for bt in range(BC // 128):
                    yt_ps = p2psum.tile([128, C], f32, tag="ytps")
                    nc.tensor.transpose(
                        out=yt_ps[:], in_=ysb[:, bt * 128:(bt + 1) * 128],
                        identity=ident[0:C, 0:C],
                    )
                    yt = p2work.tile([128, C], f32, tag="yt")
                    nc.vector.tensor_copy(out=yt[:], in_=yt_ps[:])
                    nc.sync.dma_start(out=y_d[bt * 128:(bt + 1) * 128, :], in_=yt[:])

    nc.compile()
    return nc


_CACHED = {}


def _get_program():
    if "nc" not in _CACHED:
        _CACHED["nc"] = _build_program()
    return _CACHED["nc"]


_LAST_RESULTS = None


def kernel(**inputs):
    global _LAST_RESULTS
    dev, x = _host_prep(inputs)
    nc = _get_program()

    in_maps = []
    for cid in range(NCORES):
        m = dict(dev)
        m["x_shard"] = np.ascontiguousarray(x[cid * BC:(cid + 1) * BC])
        in_maps.append(m)

    res = bass_utils.run_bass_kernel_spmd(nc, in_maps, core_ids=list(range(NCORES)))
    _LAST_RESULTS = res
    y = np.concatenate([r["y_out"] for r in res.results], axis=0)
    return y.astype(np.float32)


if __name__ == "__main__":
    # CoreSim smoke test on one core
    sys.path.insert(0, "/root/problem")
    import jax
    import reference

    with jax.default_device(jax.devices("cpu")[0]):
        inputs = {k: np.asarray(v) for k, v in reference.setup_inputs().items()}
    dev, x = _host_prep(inputs)
    nc = _build_program()
    from concourse.bass_interp import CoreSim

    sim = CoreSim(nc, trace=False)
    for k, v in dev.items():
        sim.tensor(k)[:] = v
    sim.tensor("x_shard")[:] = x[:BC]
    sim.simulate(check_with_hw=False)
    y0 = np.array(sim.tensor("y_out"))
    with jax.default_device(jax.devices("cpu")[0]):
        exp = np.asarray(reference.reference(**inputs))[:BC]
    err = np.abs(y0 - exp).max()
    rel2 = np.linalg.norm(y0 - exp) / (np.linalg.norm(exp) + 1e-30)
    print("sim maxabs:", err, " rel-l2:", rel2)

